# revision 19
# baseline (speedup 1.0000x reference)
"""Trainium2 Bass kernel for block-diagonal (chunked) causal self-attention.

Reference computation (per nn.Module):
    qkv = x @ w_attn.T; q,k,v = split(qkv)
    per (batch, head, chunk of 256 tokens): causal softmax attention in-chunk
    out = y @ w_proj.T

Sharding: the 16384 tokens (B*T) are split contiguously across 8 cores
(2048 tokens = 8 chunks per core; chunks never cross a core boundary and
attention is chunk-local, so no collectives are needed).

Per-core on-chip dataflow (matmul operands bf16, fp32 accumulation):
  xT   [1024, 2048]   x-shard transposed (feature-major)
  qkT  = wqkT.T @ xT  [2048, tok] (q rows 0:1024, k rows 1024:2048)
  v    [tok, 1024]    natural layout, [128, 16, 64] tiles
  S^T  [256k, 256q]   per (chunk, head), both k-tiles in ONE [128, 512]
                      PSUM tile = kT.T @ qT
  PT   = exp(0.125*S^T) * causal_mask  (one ACT exp + one DVE mul, bf16;
                      exp without max-subtraction is safe: scores ~ +-2)
  y^T  per HEAD PAIR into one [128, 256] PSUM tile: even head -> rows 0:64
       (tile_position (0,0)), odd head -> rows 64:128 (col-group packing,
       tile_position (0,64)), so the yT pair tile is written without any
       partition shift (DVE lanes are partition-locked) and the output
       projection contracts K=128.
  l    softmax denominators via ones-row matmuls into a [1, 512] PSUM tile
       (l = colsum(PT)); linv = exp(-ln(l)) on ACT (DVE reciprocal is
       ~7 cyc/elem - too slow), lane-broadcast via a DRAM bounce DMA.
  yT   [128, tok] per head-pair, normalized in place, then
  out  = sum_k yTpair_k.T @ wpPair_k  [tok, 1024] fp32

4 phases of 512 tokens, double-buffered. Two levels of software pipelining
keep the in-order PE stream dense (HAM clock gate: sparse stretches
re-throttle the PE to 1.2 GHz): S^T matmuls run PIPE pair-blocks ahead of
the PV matmuls, and dense projection matmul units (next phase's qkv
projection, deferred output projections) are interleaved between attention
blocks from a paced filler queue.
"""
import sys

if '/opt/trn_rl_repo' not in sys.path:
    sys.path.insert(0, '/opt/trn_rl_repo')

import numpy as np
import ml_dtypes

import concourse.bass as bass
import concourse.mybir as mybir
import concourse.tile as tile
from concourse.bass_utils import run_bass_kernel_spmd

# problem shape (hardcoded per spec)
B, T, D, H, CS = 4, 4096, 1024, 16, 256
DH = D // H            # 64
NCORES = 8
TOK = (B * T) // NCORES   # 2048 tokens per core
PH_TOK = 512              # tokens per phase
NPH = TOK // PH_TOK       # 4 phases
CPP = PH_TOK // CS        # 2 chunks per phase
MT = PH_TOK // 128        # 4 token tiles per phase
KD = D // 128             # 8 feature k-tiles
HP = H // 2               # 8 head pairs
PIPE = 3                  # attention pair-block software-pipeline depth

F32 = mybir.dt.float32
BF16 = mybir.dt.bfloat16
EXP = mybir.ActivationFunctionType.Exp
LN = mybir.ActivationFunctionType.Ln


def _split_excess_waits(nc, max_waits=1):
    """This container's walrus accepts at most one sync-wait per instruction;
    the Tile tail drain is emitted post-legalize with one wait per live proc.
    Hoist excess waits onto standalone EventSemaphore instructions."""
    for f in nc.m.functions:
        for bb in f.blocks:
            new_insts = []
            for ins in bb.instructions:
                si = ins.sync_info
                waits = list(si.on_wait) if si is not None and si.on_wait else []
                if len(waits) > max_waits:
                    for i, w in enumerate(waits[:-max_waits]):
                        ev = mybir.InstEventSemaphore(
                            name=f"{ins.name}_wsplit{i}", engine=ins.engine,
                            ins=[], outs=[],
                            sync_info=mybir.SyncInfo(on_wait=[w], on_update=[]))
                        new_insts.append(ev)
                    si.on_wait = waits[-max_waits:]
                new_insts.append(ins)
            bb.instructions = new_insts


def _build_nc():
    nc = bass.Bass()
    xT = nc.declare_dram_parameter("xT", [D, TOK], BF16, isOutput=False)
    wqkT = nc.declare_dram_parameter("wqkT", [D, 2 * D], BF16, isOutput=False)
    wvT = nc.declare_dram_parameter("wvT", [D, D], BF16, isOutput=False)
    wpT = nc.declare_dram_parameter("wpT", [D, D], BF16, isOutput=False)
    masks = nc.declare_dram_parameter("masks", [128, 2 * CS], BF16, isOutput=False)
    out = nc.declare_dram_parameter("out", [TOK, D], F32, isOutput=True)

    with tile.TileContext(nc) as tc:
        with tc.tile_pool(name="wpool", bufs=1) as wpool, \
             tc.tile_pool(name="ph", bufs=2) as ph, \
             tc.tile_pool(name="phy", bufs=2) as phy, \
             tc.tile_pool(name="wk", bufs=4) as wk, \
             tc.tile_pool(name="dr", bufs=12, space="DRAM") as dr, \
             tc.tile_pool(name="pmm", bufs=2, space="PSUM") as pmm, \
             tc.tile_pool(name="pst", bufs=3, space="PSUM") as pst, \
             tc.tile_pool(name="py", bufs=2, space="PSUM") as py, \
             tc.tile_pool(name="pl", bufs=1, space="PSUM") as pl:

            # ---- static weights ----
            # qk weights load first (after the x slice): the first
            # projection matmuls depend only on them, so the PE starts
            # ~20us earlier than if all weights queued ahead.
            wqk_pend = []
            for k in range(KD):
                t = wpool.tile([128, 2 * D], BF16, name=f"wqk{k}")
                wqk_pend.append(t)
            wqk_sb = wqk_pend

            def load_wqk():
                # the first projection matmul needs only wqk[0] + xk[0]:
                # emit weight DMAs in k order, alternating the two HWDGE
                # issue engines (SP / ACT), and split each tile in half so
                # the k=0 deps complete as early as possible
                for k in range(KD):
                    eng = nc.sync if k % 2 == 0 else nc.scalar
                    half = D
                    eng.dma_start(out=wqk_sb[k][:, 0:half],
                                  in_=wqkT[k * 128:(k + 1) * 128, 0:half])
                    eng2 = nc.scalar if k % 2 == 0 else nc.sync
                    eng2.dma_start(out=wqk_sb[k][:, half:2 * D],
                                   in_=wqkT[k * 128:(k + 1) * 128, half:2 * D])

            wv_sb = []
            wp_sb = []

            def load_late_weights():
                for k in range(KD):
                    t = wpool.tile([128, D], BF16, name=f"wv{k}")
                    nc.sync.dma_start(out=t, in_=wvT[k * 128:(k + 1) * 128, :])
                    wv_sb.append(t)
                for k in range(KD):   # head-PAIR tiles [128, D]
                    t = wpool.tile([128, D], BF16, name=f"wp{k}")
                    nc.sync.dma_start(out=t, in_=wpT[k * 128:(k + 1) * 128, :])
                    wp_sb.append(t)

            msk = wpool.tile([128, 2 * CS], BF16, name="msk")
            nc.sync.dma_start(out=msk, in_=masks[:, :])
            ones = wpool.tile([128, 1], BF16, name="ones")
            nc.gpsimd.memset(ones, 1.0)

            def load_x(p):
                xk = []
                for k in range(KD):
                    t = ph.tile([128, PH_TOK], BF16, name=f"xk{k}", tag=f"xk{k}")
                    nc.sync.dma_start(
                        out=t,
                        in_=xT[k * 128:(k + 1) * 128,
                               p * PH_TOK:(p + 1) * PH_TOK])
                    xk.append(t)
                return xk

            def qk_unit(p, xk, f):
                """One qk-projection feature tile: 8 matmuls + 1 copy."""
                ps_ = pmm.tile([128, PH_TOK], F32, name="psmm", tag="mm")
                for k in range(KD):
                    nc.tensor.matmul(
                        ps_, wqk_sb[k][:, f * 128:(f + 1) * 128], xk[k],
                        start=(k == 0), stop=(k == KD - 1))
                t = ph.tile([128, PH_TOK], BF16, name=f"qk{f}", tag=f"qk{f}")
                nc.vector.tensor_copy(out=t, in_=ps_)
                return t

            def v_unit(p, xk, vp_sb, m, n2):
                """Half of one v token-tile: 8 matmuls + strided copy."""
                if n2 == 0:
                    t = ph.tile([128, H, DH], BF16, name=f"vp{m}", tag=f"vp{m}")
                    vp_sb[m] = t
                t = vp_sb[m]
                ps_ = pmm.tile([128, 512], F32, name="psmm", tag="mm")
                for k in range(KD):
                    nc.tensor.matmul(
                        ps_, xk[k][:, m * 128:(m + 1) * 128],
                        wv_sb[k][:, n2 * 512:(n2 + 1) * 512],
                        start=(k == 0), stop=(k == KD - 1))
                nc.vector.tensor_copy(
                    out=t[:, n2 * 8:(n2 + 1) * 8, :],
                    in_=ps_.rearrange("p (h d) -> p h d", d=DH))

            def stage1(qk_sb, c, h):
                """S^T matmuls into one [128,512] psum, exp, causal mask."""
                col0 = c * CS
                ft, rh = h // 2, (h % 2) * 64
                qT = qk_sb[ft][rh:rh + 64, col0:col0 + CS]
                kT = qk_sb[KD + ft][rh:rh + 64, col0:col0 + CS]
                st = pst.tile([128, 2 * CS], F32, name="psst", tag="st")
                for kk in range(2):
                    nc.tensor.matmul(
                        st[:, kk * CS:(kk + 1) * CS],
                        kT[:, kk * 128:(kk + 1) * 128], qT,
                        start=True, stop=True)
                pt = wk.tile([128, 2 * CS], BF16, name="pt", tag="pt", bufs=6)
                nc.scalar.activation(out=pt, in_=st, func=EXP, scale=0.125)
                nc.vector.tensor_mul(pt, pt, msk)
                return pt

            def stage2(vp_sb, yT_sb, lscr_hc, c, hp, pt_e, pt_o):
                """Paired PV matmuls (even head -> psum rows 0:64, odd head
                -> rows 64:128 via col-group packing) + denominator matmuls
                (l = colsum(PT) via a ones row). linv = exp(-ln(l)) on ACT
                is DMA'd to a DRAM collector for the batched lane
                broadcast; unnormalized y^T pair goes straight to yT."""
                col0 = c * CS
                ps_y = py.tile([128, CS], F32, name="psy", tag="y")
                for kk in range(2):
                    vsl_e = vp_sb[CPP * c + kk][:, 2 * hp, :]
                    nc.tensor.matmul(ps_y[0:64, :], vsl_e,
                                     pt_e[:, kk * CS:(kk + 1) * CS],
                                     start=(kk == 0), stop=(kk == 1),
                                     tile_position=(0, 0))
                for kk in range(2):
                    vsl_o = vp_sb[CPP * c + kk][:, 2 * hp + 1, :]
                    nc.tensor.matmul(ps_y[64:128, :], vsl_o,
                                     pt_o[:, kk * CS:(kk + 1) * CS],
                                     start=(kk == 0), stop=(kk == 1),
                                     tile_position=(0, 64))
                ps_l = pl.tile([1, 2 * CS], F32, name="psl", tag="l")
                for kk in range(2):
                    nc.tensor.matmul(ps_l[:, 0:CS], ones,
                                     pt_e[:, kk * CS:(kk + 1) * CS],
                                     start=(kk == 0), stop=(kk == 1))
                for kk in range(2):
                    nc.tensor.matmul(ps_l[:, CS:2 * CS], ones,
                                     pt_o[:, kk * CS:(kk + 1) * CS],
                                     start=(kk == 0), stop=(kk == 1))
                lnl = wk.tile([1, 2 * CS], F32, name="lnl", tag="lnl", bufs=4)
                nc.scalar.activation(out=lnl, in_=ps_l, func=LN)
                nc.vector.tensor_copy(out=yT_sb[hp][:, col0:col0 + CS],
                                      in_=ps_y)
                linv = wk.tile([1, 2 * CS], F32, name="linv", tag="linv",
                               bufs=4)
                nc.scalar.activation(out=linv, in_=lnl, func=EXP, scale=-1.0)
                nc.sync.dma_start(
                    out=lscr_hc[:, (hp % 4) * 2 * CS:(hp % 4 + 1) * 2 * CS],
                    in_=linv)

            def halfchunk_norm(yT_sb, lscr_hc, c, hp0):
                """Broadcast 4 pairs' linv rows across partitions via a
                DRAM-bounce DMA (engines are partition-locked; DMA is the
                lane shuffle), then normalize their yT slices in place.
                rrep layout [128, 4, 256]: rows 0:64 even-head linv, rows
                64:128 odd-head linv, matching the yT pair layout."""
                col0 = c * CS
                rrep = wk.tile([128, 4, CS], F32, name="rrep", tag="rrep",
                               bufs=2)
                for par in range(2):   # even rows / odd rows
                    bc = bass.AP(tensor=lscr_hc.tensor,
                                 offset=lscr_hc.offset + par * CS,
                                 ap=[[0, 64], [2 * CS, 4], [1, CS]])
                    nc.sync.dma_start(out=rrep[par * 64:(par + 1) * 64],
                                      in_=bc)
                for i in range(4):
                    ysl = yT_sb[hp0 + i][:, col0:col0 + CS]
                    nc.vector.tensor_mul(ysl, ysl, rrep[:, i, :])

            def out_unit(p, yT_sb, m, n):
                ps_ = pmm.tile([128, 512], F32, name="psmm", tag="mm")
                for k in range(KD):
                    nc.tensor.matmul(
                        ps_, yT_sb[k][:, m * 128:(m + 1) * 128],
                        wp_sb[k][:, n * 512:(n + 1) * 512],
                        start=(k == 0), stop=(k == KD - 1))
                ost = wk.tile([128, 512], F32, name="ost", tag="ost", bufs=3)
                nc.vector.tensor_copy(out=ost, in_=ps_)
                nc.sync.dma_start(
                    out=out[p * PH_TOK + m * 128: p * PH_TOK + (m + 1) * 128,
                            n * 512:(n + 1) * 512],
                    in_=ost)

            # ---- prologue: phase 0 projections ----
            # x slice is small (1 MiB) - load it before the 4 MiB qk
            # weights so the first matmul's deps land ASAP.
            xk_cur = load_x(0)
            load_wqk()
            qk_cur = [qk_unit(0, xk_cur, f) for f in range(4)]
            load_late_weights()   # v/out weights DMA behind the first MMs
            qk_cur += [qk_unit(0, xk_cur, f) for f in range(4, 2 * KD)]
            vp_cur = [None] * MT
            for m in range(MT):
                for n2 in range(2):
                    v_unit(0, xk_cur, vp_cur, m, n2)

            deferred = []   # out-proj units of phase p-1, run in phase p
            for p in range(NPH):
                yT_sb = [phy.tile([128, PH_TOK], BF16, name=f"yT{j}",
                                  tag=f"yT{j}") for j in range(HP)]
                # filler queue: dense projection matmul units interleaved
                # between attention blocks to keep HAM at K=8/8
                filler = list(deferred)
                deferred = []
                if p + 1 < NPH:
                    xk_nxt = load_x(p + 1)
                    qk_nxt = [None] * (2 * KD)
                    vp_nxt = [None] * MT

                    def mk_qk(f):
                        return lambda: qk_nxt.__setitem__(
                            f, qk_unit(p + 1, xk_nxt, f))

                    def mk_v(m, n2):
                        return lambda: v_unit(p + 1, xk_nxt, vp_nxt, m, n2)

                    filler += [mk_qk(f) for f in range(2 * KD)]
                    filler += [mk_v(m, n2) for m in range(MT)
                               for n2 in range(2)]

                def mk_out(m, n):
                    def go(m=m, n=n, yts=yT_sb, p0=p):
                        out_unit(p0, yts, m, n)
                    return go

                lscrs = [dr.tile([1, 4 * 2 * CS], F32, name=f"lscr{i}",
                                 tag="lscr", bufs=4)
                         for i in range(CPP * 2)]
                pending = []
                done_s2 = 0

                def flush_one():
                    nonlocal done_s2
                    c2, hp2, pte, pto = pending.pop(0)
                    hc = c2 * 2 + (hp2 // 4)
                    stage2(vp_cur, yT_sb, lscrs[hc], c2, hp2, pte, pto)
                    done_s2 += 1
                    if done_s2 % 4 == 0:
                        hcd = done_s2 // 4 - 1
                        halfchunk_norm(yT_sb, lscrs[hcd], hcd // 2,
                                       (hcd % 2) * 4)
                        if hcd == 1:     # chunk 0 normalized
                            filler.extend([mk_out(mm_, nn_)
                                           for mm_ in range(2)
                                           for nn_ in range(2)])
                        elif hcd == 3:   # chunk 1 normalized
                            units = [mk_out(mm_, nn_)
                                     for mm_ in range(2, MT)
                                     for nn_ in range(2)]
                            if p + 1 < NPH:
                                deferred.extend(units)
                            else:
                                filler.extend(units)

                nblocks = CPP * HP
                bi = 0
                for c in range(CPP):
                    for hp in range(HP):
                        pt_e = stage1(qk_cur, c, 2 * hp)
                        pt_o = stage1(qk_cur, c, 2 * hp + 1)
                        pending.append((c, hp, pt_e, pt_o))
                        # paced filler: spread queue over remaining blocks
                        left = nblocks - bi
                        take = max(1 if filler else 0,
                                   (len(filler) + left - 1) // left)
                        for _ in range(min(take, len(filler))):
                            filler.pop(0)()
                        if len(pending) > PIPE:
                            flush_one()
                        bi += 1
                while pending:
                    if filler:
                        filler.pop(0)()
                    flush_one()
                while filler:
                    filler.pop(0)()
                if p + 1 < NPH:
                    xk_cur, qk_cur, vp_cur = xk_nxt, qk_nxt, vp_nxt

    _split_excess_waits(nc)
    return nc


_NC_CACHE = None


def _get_nc():
    global _NC_CACHE
    if _NC_CACHE is None:
        _NC_CACHE = _build_nc()
    return _NC_CACHE


def _prep_shared(w_attn, w_proj):
    wqkT = np.ascontiguousarray(w_attn[:2 * D, :].T).astype(ml_dtypes.bfloat16)
    wvT = np.ascontiguousarray(w_attn[2 * D:, :].T).astype(ml_dtypes.bfloat16)
    wpT = np.ascontiguousarray(w_proj.T).astype(ml_dtypes.bfloat16)
    ii = np.arange(128)[:, None]
    qq = np.arange(CS)[None, :]
    masks = np.concatenate([(ii <= qq), (ii + 128 <= qq)],
                           axis=1).astype(ml_dtypes.bfloat16)   # [128, 512]
    return wqkT, wvT, wpT, masks


def kernel(x, w_attn, w_proj, _trace=False):
    x = np.asarray(x)
    w_attn = np.asarray(w_attn)
    w_proj = np.asarray(w_proj)
    wqkT, wvT, wpT, masks = _prep_shared(w_attn, w_proj)
    x_flat = x.reshape(B * T, D)
    in_maps = []
    for c in range(NCORES):
        xTc = np.ascontiguousarray(
            x_flat[c * TOK:(c + 1) * TOK, :].T).astype(ml_dtypes.bfloat16)
        in_maps.append({"xT": xTc, "wqkT": wqkT, "wvT": wvT, "wpT": wpT,
                        "masks": masks})
    nc = _get_nc()
    kw = {}
    if _trace:
        kw["trace"] = True
    res = run_bass_kernel_spmd(nc, in_maps, core_ids=list(range(NCORES)), **kw)
    outs = [res.results[c]["out"] for c in range(NCORES)]
    full = np.concatenate(outs, axis=0).reshape(B, T, D)
    if _trace:
        return full, res
    return full


# revision 20
# speedup vs baseline: 1.0866x; 1.0866x over previous
"""Trainium2 Bass kernel for block-diagonal (chunked) causal self-attention.

Reference computation (per nn.Module):
    qkv = x @ w_attn.T; q,k,v = split(qkv)
    per (batch, head, chunk of 256 tokens): causal softmax attention in-chunk
    out = y @ w_proj.T

Sharding: the 16384 tokens (B*T) are split contiguously across 8 cores
(2048 tokens = 8 chunks per core; chunks never cross a core boundary and
attention is chunk-local, so no collectives are needed).

Per-core on-chip dataflow (matmul operands bf16, fp32 accumulation):
  xT   [1024, 2048]   x-shard transposed (feature-major)
  qkT  = wqkT.T @ xT  [2048, tok] (q rows 0:1024, k rows 1024:2048)
  v    [tok, 1024]    natural layout, [128, 16, 64] tiles
  S^T  [256k, 256q]   per (chunk, head), both k-tiles in ONE [128, 512]
                      PSUM tile = kT.T @ qT
  PT   = exp(0.125*S^T) * causal_mask  (one ACT exp + one DVE mul, bf16;
                      exp without max-subtraction is safe: scores ~ +-2)
  y^T  per HEAD PAIR into one [128, 256] PSUM tile: even head -> rows 0:64
       (tile_position (0,0)), odd head -> rows 64:128 (col-group packing,
       tile_position (0,64)), so the yT pair tile is written without any
       partition shift (DVE lanes are partition-locked) and the output
       projection contracts K=128.
  l    softmax denominators via ones-row matmuls into a [1, 512] PSUM tile
       (l = colsum(PT)); linv = exp(-ln(l)) on ACT (DVE reciprocal is
       ~7 cyc/elem - too slow), lane-broadcast via a DRAM bounce DMA.
  yT   [128, tok] per head-pair, normalized in place, then
  out  = sum_k yTpair_k.T @ wpPair_k  [tok, 1024] fp32

4 phases of 512 tokens, double-buffered. Two levels of software pipelining
keep the in-order PE stream dense (HAM clock gate: sparse stretches
re-throttle the PE to 1.2 GHz): S^T matmuls run PIPE pair-blocks ahead of
the PV matmuls, and dense projection matmul units (next phase's qkv
projection, deferred output projections) are interleaved between attention
blocks from a paced filler queue.
"""
import sys

if '/opt/trn_rl_repo' not in sys.path:
    sys.path.insert(0, '/opt/trn_rl_repo')

import numpy as np
import ml_dtypes

import concourse.bass as bass
import concourse.mybir as mybir
import concourse.tile as tile
from concourse.bass_utils import run_bass_kernel_spmd

# problem shape (hardcoded per spec)
B, T, D, H, CS = 4, 4096, 1024, 16, 256
DH = D // H            # 64
NCORES = 8
TOK = (B * T) // NCORES   # 2048 tokens per core
PH_TOK = 512              # tokens per phase
NPH = TOK // PH_TOK       # 4 phases
CPP = PH_TOK // CS        # 2 chunks per phase
MT = PH_TOK // 128        # 4 token tiles per phase
KD = D // 128             # 8 feature k-tiles
HP = H // 2               # 8 head pairs
PIPE = 2                  # attention pair-block software-pipeline depth

F32 = mybir.dt.float32
BF16 = mybir.dt.bfloat16
EXP = mybir.ActivationFunctionType.Exp
LN = mybir.ActivationFunctionType.Ln


def _split_excess_waits(nc, max_waits=1):
    """This container's walrus accepts at most one sync-wait per instruction;
    the Tile tail drain is emitted post-legalize with one wait per live proc.
    Hoist excess waits onto standalone EventSemaphore instructions."""
    for f in nc.m.functions:
        for bb in f.blocks:
            new_insts = []
            for ins in bb.instructions:
                si = ins.sync_info
                waits = list(si.on_wait) if si is not None and si.on_wait else []
                if len(waits) > max_waits:
                    for i, w in enumerate(waits[:-max_waits]):
                        ev = mybir.InstEventSemaphore(
                            name=f"{ins.name}_wsplit{i}", engine=ins.engine,
                            ins=[], outs=[],
                            sync_info=mybir.SyncInfo(on_wait=[w], on_update=[]))
                        new_insts.append(ev)
                    si.on_wait = waits[-max_waits:]
                new_insts.append(ins)
            bb.instructions = new_insts


def _build_nc():
    nc = bass.Bass()
    xT = nc.declare_dram_parameter("xT", [D, TOK], BF16, isOutput=False)
    wqkT = nc.declare_dram_parameter("wqkT", [D, 2 * D], BF16, isOutput=False)
    wvT = nc.declare_dram_parameter("wvT", [D, D], BF16, isOutput=False)
    wpT = nc.declare_dram_parameter("wpT", [D, D], BF16, isOutput=False)
    masks = nc.declare_dram_parameter("masks", [128, 2 * CS], BF16, isOutput=False)
    out = nc.declare_dram_parameter("out", [TOK, D], F32, isOutput=True)

    with tile.TileContext(nc) as tc:
        with tc.tile_pool(name="wpool", bufs=1) as wpool, \
             tc.tile_pool(name="ph", bufs=2) as ph, \
             tc.tile_pool(name="phy", bufs=2) as phy, \
             tc.tile_pool(name="wk", bufs=4) as wk, \
             tc.tile_pool(name="dr", bufs=12, space="DRAM") as dr, \
             tc.tile_pool(name="pmm", bufs=2, space="PSUM") as pmm, \
             tc.tile_pool(name="pst", bufs=3, space="PSUM") as pst, \
             tc.tile_pool(name="py", bufs=2, space="PSUM") as py, \
             tc.tile_pool(name="pl", bufs=1, space="PSUM") as pl:

            # ---- static weights ----
            # qk weights load first (after the x slice): the first
            # projection matmuls depend only on them, so the PE starts
            # ~20us earlier than if all weights queued ahead.
            wqk_pend = []
            for k in range(KD):
                t = wpool.tile([128, 2 * D], BF16, name=f"wqk{k}")
                wqk_pend.append(t)
            wqk_sb = wqk_pend

            def load_wqk():
                # the first projection matmul needs only wqk[0] + xk[0]:
                # emit weight DMAs in k order, alternating the two HWDGE
                # issue engines (SP / ACT), and split each tile in half so
                # the k=0 deps complete as early as possible
                for k in range(KD):
                    eng = nc.sync if k % 2 == 0 else nc.scalar
                    half = D
                    eng.dma_start(out=wqk_sb[k][:, 0:half],
                                  in_=wqkT[k * 128:(k + 1) * 128, 0:half])
                    eng2 = nc.scalar if k % 2 == 0 else nc.sync
                    eng2.dma_start(out=wqk_sb[k][:, half:2 * D],
                                   in_=wqkT[k * 128:(k + 1) * 128, half:2 * D])

            wv_sb = []
            wp_sb = []

            def load_late_weights():
                for k in range(KD):
                    t = wpool.tile([128, D], BF16, name=f"wv{k}")
                    nc.sync.dma_start(out=t, in_=wvT[k * 128:(k + 1) * 128, :])
                    wv_sb.append(t)
                for k in range(KD):   # head-PAIR tiles [128, D]
                    t = wpool.tile([128, D], BF16, name=f"wp{k}")
                    nc.sync.dma_start(out=t, in_=wpT[k * 128:(k + 1) * 128, :])
                    wp_sb.append(t)

            msk = wpool.tile([128, 2 * CS], BF16, name="msk")
            nc.sync.dma_start(out=msk, in_=masks[:, :])
            ones = wpool.tile([128, 1], BF16, name="ones")
            nc.gpsimd.memset(ones, 1.0)

            def load_x(p):
                xk = []
                for k in range(KD):
                    t = ph.tile([128, PH_TOK], BF16, name=f"xk{k}", tag=f"xk{k}")
                    nc.sync.dma_start(
                        out=t,
                        in_=xT[k * 128:(k + 1) * 128,
                               p * PH_TOK:(p + 1) * PH_TOK])
                    xk.append(t)
                return xk

            def qk_unit(p, xk, f):
                """One qk-projection feature tile: 8 matmuls + 1 copy."""
                ps_ = pmm.tile([128, PH_TOK], F32, name="psmm", tag="mm")
                for k in range(KD):
                    nc.tensor.matmul(
                        ps_, wqk_sb[k][:, f * 128:(f + 1) * 128], xk[k],
                        start=(k == 0), stop=(k == KD - 1))
                t = ph.tile([128, PH_TOK], BF16, name=f"qk{f}", tag=f"qk{f}")
                nc.vector.tensor_copy(out=t, in_=ps_)
                return t

            def v_unit(p, xk, vp_sb, m, n2):
                """Half of one v token-tile: 8 matmuls + strided copy."""
                if n2 == 0:
                    t = ph.tile([128, H, DH], BF16, name=f"vp{m}", tag=f"vp{m}")
                    vp_sb[m] = t
                t = vp_sb[m]
                ps_ = pmm.tile([128, 512], F32, name="psmm", tag="mm")
                for k in range(KD):
                    nc.tensor.matmul(
                        ps_, xk[k][:, m * 128:(m + 1) * 128],
                        wv_sb[k][:, n2 * 512:(n2 + 1) * 512],
                        start=(k == 0), stop=(k == KD - 1))
                nc.vector.tensor_copy(
                    out=t[:, n2 * 8:(n2 + 1) * 8, :],
                    in_=ps_.rearrange("p (h d) -> p h d", d=DH))

            def stage1(qk_sb, c, h):
                """S^T matmuls into one [128,512] psum, exp, causal mask."""
                col0 = c * CS
                ft, rh = h // 2, (h % 2) * 64
                qT = qk_sb[ft][rh:rh + 64, col0:col0 + CS]
                kT = qk_sb[KD + ft][rh:rh + 64, col0:col0 + CS]
                st = pst.tile([128, 2 * CS], F32, name="psst", tag="st")
                for kk in range(2):
                    nc.tensor.matmul(
                        st[:, kk * CS:(kk + 1) * CS],
                        kT[:, kk * 128:(kk + 1) * 128], qT,
                        start=True, stop=True)
                pt = wk.tile([128, 2 * CS], BF16, name="pt", tag="pt", bufs=6)
                nc.scalar.activation(out=pt, in_=st, func=EXP, scale=0.125)
                nc.vector.tensor_mul(pt, pt, msk)
                return pt

            def stage2(vp_sb, yT_sb, lscr_hc, c, hp, pt_e, pt_o):
                """Paired PV matmuls (even head -> psum rows 0:64, odd head
                -> rows 64:128 via col-group packing) + denominator matmuls
                (l = colsum(PT) via a ones row). linv = exp(-ln(l)) on ACT
                is DMA'd to a DRAM collector for the batched lane
                broadcast; unnormalized y^T pair goes straight to yT."""
                col0 = c * CS
                ps_y = py.tile([128, CS], F32, name="psy", tag="y")
                for kk in range(2):
                    vsl_e = vp_sb[CPP * c + kk][:, 2 * hp, :]
                    nc.tensor.matmul(ps_y[0:64, :], vsl_e,
                                     pt_e[:, kk * CS:(kk + 1) * CS],
                                     start=(kk == 0), stop=(kk == 1),
                                     tile_position=(0, 0))
                for kk in range(2):
                    vsl_o = vp_sb[CPP * c + kk][:, 2 * hp + 1, :]
                    nc.tensor.matmul(ps_y[64:128, :], vsl_o,
                                     pt_o[:, kk * CS:(kk + 1) * CS],
                                     start=(kk == 0), stop=(kk == 1),
                                     tile_position=(0, 64))
                ps_l = pl.tile([1, 2 * CS], F32, name="psl", tag="l")
                for kk in range(2):
                    nc.tensor.matmul(ps_l[:, 0:CS], ones,
                                     pt_e[:, kk * CS:(kk + 1) * CS],
                                     start=(kk == 0), stop=(kk == 1))
                for kk in range(2):
                    nc.tensor.matmul(ps_l[:, CS:2 * CS], ones,
                                     pt_o[:, kk * CS:(kk + 1) * CS],
                                     start=(kk == 0), stop=(kk == 1))
                lnl = wk.tile([1, 2 * CS], F32, name="lnl", tag="lnl", bufs=4)
                nc.scalar.activation(out=lnl, in_=ps_l, func=LN)
                nc.vector.tensor_copy(out=yT_sb[hp][:, col0:col0 + CS],
                                      in_=ps_y)
                linv = wk.tile([1, 2 * CS], F32, name="linv", tag="linv",
                               bufs=4)
                nc.scalar.activation(out=linv, in_=lnl, func=EXP, scale=-1.0)
                nc.sync.dma_start(
                    out=lscr_hc[:, (hp % 4) * 2 * CS:(hp % 4 + 1) * 2 * CS],
                    in_=linv)

            def halfchunk_norm(yT_sb, lscr_hc, c, hp0):
                """Broadcast 4 pairs' linv rows across partitions via a
                DRAM-bounce DMA (engines are partition-locked; DMA is the
                lane shuffle), then normalize their yT slices in place.
                rrep layout [128, 4, 256]: rows 0:64 even-head linv, rows
                64:128 odd-head linv, matching the yT pair layout."""
                col0 = c * CS
                rrep = wk.tile([128, 4, CS], F32, name="rrep", tag="rrep",
                               bufs=2)
                for par in range(2):   # even rows / odd rows
                    bc = bass.AP(tensor=lscr_hc.tensor,
                                 offset=lscr_hc.offset + par * CS,
                                 ap=[[0, 64], [2 * CS, 4], [1, CS]])
                    nc.sync.dma_start(out=rrep[par * 64:(par + 1) * 64],
                                      in_=bc)
                for i in range(4):
                    ysl = yT_sb[hp0 + i][:, col0:col0 + CS]
                    nc.vector.tensor_mul(ysl, ysl, rrep[:, i, :])

            def out_unit(p, yT_sb, m, n):
                ps_ = pmm.tile([128, 512], F32, name="psmm", tag="mm")
                for k in range(KD):
                    nc.tensor.matmul(
                        ps_, yT_sb[k][:, m * 128:(m + 1) * 128],
                        wp_sb[k][:, n * 512:(n + 1) * 512],
                        start=(k == 0), stop=(k == KD - 1))
                ost = wk.tile([128, 512], F32, name="ost", tag="ost", bufs=3)
                nc.vector.tensor_copy(out=ost, in_=ps_)
                nc.sync.dma_start(
                    out=out[p * PH_TOK + m * 128: p * PH_TOK + (m + 1) * 128,
                            n * 512:(n + 1) * 512],
                    in_=ost)

            # ---- prologue: phase 0 projections ----
            # x slice is small (1 MiB) - load it before the 4 MiB qk
            # weights so the first matmul's deps land ASAP.
            xk_cur = load_x(0)
            load_wqk()
            qk_cur = [qk_unit(0, xk_cur, f) for f in range(4)]
            load_late_weights()   # v/out weights DMA behind the first MMs
            qk_cur += [qk_unit(0, xk_cur, f) for f in range(4, 2 * KD)]
            vp_cur = [None] * MT
            for m in range(MT):
                for n2 in range(2):
                    v_unit(0, xk_cur, vp_cur, m, n2)

            deferred = []   # out-proj units of phase p-1, run in phase p
            for p in range(NPH):
                yT_sb = [phy.tile([128, PH_TOK], BF16, name=f"yT{j}",
                                  tag=f"yT{j}") for j in range(HP)]
                # filler queue: dense projection matmul units interleaved
                # between attention blocks to keep HAM at K=8/8
                filler = list(deferred)
                deferred = []
                if p + 1 < NPH:
                    xk_nxt = load_x(p + 1)
                    qk_nxt = [None] * (2 * KD)
                    vp_nxt = [None] * MT

                    def mk_qk(f):
                        return lambda: qk_nxt.__setitem__(
                            f, qk_unit(p + 1, xk_nxt, f))

                    def mk_v(m, n2):
                        return lambda: v_unit(p + 1, xk_nxt, vp_nxt, m, n2)

                    filler += [mk_qk(f) for f in range(2 * KD)]
                    filler += [mk_v(m, n2) for m in range(MT)
                               for n2 in range(2)]

                def mk_out(m, n):
                    def go(m=m, n=n, yts=yT_sb, p0=p):
                        out_unit(p0, yts, m, n)
                    return go

                lscrs = [dr.tile([1, 4 * 2 * CS], F32, name=f"lscr{i}",
                                 tag="lscr", bufs=4)
                         for i in range(CPP * 2)]
                pending = []
                done_s2 = 0

                def flush_one():
                    nonlocal done_s2
                    c2, hp2, pte, pto = pending.pop(0)
                    hc = c2 * 2 + (hp2 // 4)
                    stage2(vp_cur, yT_sb, lscrs[hc], c2, hp2, pte, pto)
                    done_s2 += 1
                    if done_s2 % 4 == 0:
                        hcd = done_s2 // 4 - 1
                        halfchunk_norm(yT_sb, lscrs[hcd], hcd // 2,
                                       (hcd % 2) * 4)
                        if hcd == 1:     # chunk 0 normalized
                            filler.extend([mk_out(mm_, nn_)
                                           for mm_ in range(2)
                                           for nn_ in range(2)])
                        elif hcd == 3:   # chunk 1 normalized
                            units = [mk_out(mm_, nn_)
                                     for mm_ in range(2, MT)
                                     for nn_ in range(2)]
                            if p + 1 < NPH:
                                deferred.extend(units)
                            else:
                                filler.extend(units)

                nblocks = CPP * HP
                bi = 0
                for c in range(CPP):
                    for hp in range(HP):
                        pt_e = stage1(qk_cur, c, 2 * hp)
                        pt_o = stage1(qk_cur, c, 2 * hp + 1)
                        pending.append((c, hp, pt_e, pt_o))
                        # paced filler: spread queue over remaining blocks
                        left = nblocks - bi
                        take = max(1 if filler else 0,
                                   (len(filler) + left - 1) // left)
                        for _ in range(min(take, len(filler))):
                            filler.pop(0)()
                        if len(pending) > PIPE:
                            flush_one()
                        bi += 1
                while pending:
                    if filler:
                        filler.pop(0)()
                    flush_one()
                while filler:
                    filler.pop(0)()
                if p + 1 < NPH:
                    xk_cur, qk_cur, vp_cur = xk_nxt, qk_nxt, vp_nxt

    _split_excess_waits(nc)
    return nc


_NC_CACHE = None


def _get_nc():
    global _NC_CACHE
    if _NC_CACHE is None:
        _NC_CACHE = _build_nc()
    return _NC_CACHE


def _prep_shared(w_attn, w_proj):
    wqkT = np.ascontiguousarray(w_attn[:2 * D, :].T).astype(ml_dtypes.bfloat16)
    wvT = np.ascontiguousarray(w_attn[2 * D:, :].T).astype(ml_dtypes.bfloat16)
    wpT = np.ascontiguousarray(w_proj.T).astype(ml_dtypes.bfloat16)
    ii = np.arange(128)[:, None]
    qq = np.arange(CS)[None, :]
    masks = np.concatenate([(ii <= qq), (ii + 128 <= qq)],
                           axis=1).astype(ml_dtypes.bfloat16)   # [128, 512]
    return wqkT, wvT, wpT, masks


def kernel(x, w_attn, w_proj, _trace=False):
    x = np.asarray(x)
    w_attn = np.asarray(w_attn)
    w_proj = np.asarray(w_proj)
    wqkT, wvT, wpT, masks = _prep_shared(w_attn, w_proj)
    x_flat = x.reshape(B * T, D)
    in_maps = []
    for c in range(NCORES):
        xTc = np.ascontiguousarray(
            x_flat[c * TOK:(c + 1) * TOK, :].T).astype(ml_dtypes.bfloat16)
        in_maps.append({"xT": xTc, "wqkT": wqkT, "wvT": wvT, "wpT": wpT,
                        "masks": masks})
    nc = _get_nc()
    kw = {}
    if _trace:
        kw["trace"] = True
    res = run_bass_kernel_spmd(nc, in_maps, core_ids=list(range(NCORES)), **kw)
    outs = [res.results[c]["out"] for c in range(NCORES)]
    full = np.concatenate(outs, axis=0).reshape(B, T, D)
    if _trace:
        return full, res
    return full
